# revision 1
# baseline (speedup 1.0000x reference)
"""Trainium2 Bass kernel for nn_DMPNN_Change_678604832935 (8-core SPMD DMPNN+Set2Set).

Sharding: each core owns 64 consecutive graphs (batch is sorted) plus all edges
whose dst node falls in those graphs — so segment_sum is core-local and no
collectives are needed.  The node-side product z = relu(x@W0+b0) @ Wm1[:D] is
computed replicated on every core into its own HBM; per-edge rows are fetched
with dma_gather.  Since segment_sum is linear and sits between the Wm2 matmul
and the root update, m@Wm2 is folded to the node side (16x fewer FLOPs), with
deg(n)*bm2 as a rank-1 correction.  Set2Set runs in a [graph, 128-slot] grid
layout; softmax uses unnormalized exp (|e|<~8, validated) and sigmoid is
synthesized from tanh so the whole kernel uses one ACT table set.
"""

import os
import sys

for _p in ("/opt/trn_rl_repo", "/root/.axon_site/_ro/trn_rl_repo"):
    if os.path.isdir(_p) and _p not in sys.path:
        sys.path.append(_p)

import numpy as np

import concourse.bass as bass
import concourse.bacc as bacc
import concourse.mybir as mybir
import concourse.tile as tile
from concourse.bass_utils import run_bass_kernel_spmd

F16 = mybir.dt.float16
F32 = mybir.dt.float32
I16 = mybir.dt.int16
AF = mybir.ActivationFunctionType
ALU = mybir.AluOpType

N_NODES = 30000
FIN = 25
FE = 14
D = 256
N_GRAPHS = 512
N_CORES = 8
GPC = N_GRAPHS // N_CORES      # graphs per core
SLOT = 128                     # grid slots per graph
GRID = GPC * SLOT              # grid rows per core
NCH = GRID // 128              # grid chunks per core (== GPC)
NPAD = ((N_NODES + 511) // 512) * 512
GCHUNKS = 32                   # edge chunks per input-stream DMA


def _f16(a):
    return np.ascontiguousarray(np.asarray(a, np.float32).astype(np.float16))


def _host_prep(inp):
    """Pure index/layout/dtype work: build per-core input maps."""
    x = np.asarray(inp["x"], np.float32)
    ea = np.asarray(inp["edge_attr"], np.float32)
    ei = np.asarray(inp["edge_index"])
    batch = np.asarray(inp["batch"]).astype(np.int64)
    src_all = np.asarray(ei[0], np.int64)
    dst_all = np.asarray(ei[1], np.int64)

    counts = np.bincount(batch, minlength=N_GRAPHS)
    assert counts.max() <= SLOT, f"graph larger than SLOT: {counts.max()}"
    starts = np.zeros(N_GRAPHS + 1, np.int64)
    np.cumsum(counts, out=starts[1:])

    gslot = (batch % GPC) * SLOT + (np.arange(N_NODES) - starts[batch])
    dst_core = batch[dst_all] // GPC
    dst_gslot = gslot[dst_all]

    epg = np.bincount(batch[dst_all], minlength=N_GRAPHS)
    EPC = max(1, int(np.ceil(epg.max() / 128.0)))
    NEC = NCH * EPC                    # edge chunks per core (64*EPC, %16==0)
    EP = NEC * 128

    W0 = np.asarray(inp["W0"], np.float32); b0 = np.asarray(inp["b0"], np.float32)
    Wm1 = np.asarray(inp["Wm1"], np.float32); bm1 = np.asarray(inp["bm1"], np.float32)
    Wm2 = np.asarray(inp["Wm2"], np.float32); bm2 = np.asarray(inp["bm2"], np.float32)
    Wr = np.asarray(inp["Wr"], np.float32); br = np.asarray(inp["br"], np.float32)
    Wih = np.asarray(inp["Wih"], np.float32); Whh = np.asarray(inp["Whh"], np.float32)
    bl = np.asarray(inp["bl"], np.float32)
    W1 = np.asarray(inp["W1"], np.float32); b1 = np.asarray(inp["b1"], np.float32)
    W2 = np.asarray(inp["W2"], np.float32); b2 = np.asarray(inp["b2"], np.float32)

    W0c = _f16(np.concatenate([W0, b0[None, :]], 0))            # [26, 256]
    Wm1h = _f16(Wm1[:D])
    Wm1ec = _f16(np.concatenate([Wm1[D:], bm1[None, :]], 0))    # [15, 256]
    Wih_s = Wih.copy(); Wih_s[:D] *= 0.5                        # h state kept as 2h
    W1_s = W1.copy(); W1_s[:D] *= 0.5
    W1p = np.zeros((128, 4, 2, 128), np.float16)
    for kk in range(4):
        for m in range(2):
            W1p[:, kk, m, :] = _f16(W1_s[kk * 128:(kk + 1) * 128,
                                         m * 128:(m + 1) * 128])
    b1c = np.zeros((128, 2), np.float32)
    b1c[:, 0] = b1[:128]; b1c[:, 1] = b1[128:]
    W2s = np.zeros((128, 2), np.float16)
    W2s[:, 0] = _f16(W2[:128, 0]); W2s[:, 1] = _f16(W2[128:, 0])

    shared = dict(
        W0c=W0c,
        Wm1h_hi=_f16(Wm1h[:128]), Wm1h_lo=_f16(Wm1h[128:]),
        Wm1ec=Wm1ec,
        Wm2_hi=_f16(Wm2[:128]), Wm2_lo=_f16(Wm2[128:]),
        Wr_hi=_f16(Wr[:128]), Wr_lo=_f16(Wr[128:]),
        bmbr=_f16(np.stack([bm2, br], 0)),
        Wih=np.ascontiguousarray(_f16(Wih_s).reshape(4, 128, 1024).transpose(1, 0, 2)),
        Whh=np.ascontiguousarray(_f16(Whh * 0.5).reshape(2, 128, 1024).transpose(1, 0, 2)),
        blr=_f16(bl[None, :]),
        W1p=W1p, b1c=b1c, W2s=W2s, b2t=_f16(b2.reshape(1, 1)),
        ones1=np.ones((1, 64), np.float16),
        iota_row=np.tile(np.arange(128, dtype=np.float16)[None, :], (128, 1)),
        ident=np.eye(128, dtype=np.float16),
    )

    in_maps = []
    for k in range(N_CORES):
        g0 = k * GPC
        ns, ne = int(starts[g0]), int(starts[g0 + GPC])
        nodes = np.arange(ns, ne)
        gs = gslot[nodes]
        gr = batch[nodes] - g0

        xTg = np.zeros((FIN + 1, GRID), np.float16)
        xTg[:FIN, gs] = _f16(x[nodes].T)
        xTg[FIN, :] = 1.0

        Gp = np.zeros((128, NCH * GPC), np.float16)
        Gp[gs % 128, (gs // 128) * GPC + gr] = 1.0
        GTp = np.zeros((64, GRID), np.float16)
        GTp[gr, gs] = 1.0

        m = dst_core == k
        e_src = src_all[m]; e_slot = dst_gslot[m]; e_ea = ea[m]
        e_chunk = e_slot // 128
        order = np.argsort(e_chunk, kind="stable")
        e_src, e_slot, e_ea = e_src[order], e_slot[order], e_ea[order]
        e_chunk = e_chunk[order]

        deg = np.zeros(GRID, np.float32)
        np.add.at(deg, e_slot, 1.0)
        degones = np.zeros((2, GRID), np.float16)
        degones[0] = deg.astype(np.float16); degones[1] = 1.0

        srcp = np.zeros(EP, np.int64)
        colp = np.full(EP, 255.0, np.float32)
        eap = np.zeros((EP, FE + 1), np.float16)
        cstart = np.searchsorted(e_chunk, np.arange(NCH + 1))
        for c in range(NCH):
            a, b = int(cstart[c]), int(cstart[c + 1])
            n_e = b - a
            assert n_e <= EPC * 128
            o = c * EPC * 128
            srcp[o:o + n_e] = e_src[a:b]
            colp[o:o + n_e] = (e_slot[a:b] % 128).astype(np.float32)
            eap[o:o + n_e, :FE] = _f16(e_ea[a:b])
            eap[o:o + n_e, FE] = 1.0

        xgT = np.empty((FIN + 1, EP), np.float16)
        xgT[:FIN] = _f16(x[srcp].T)
        xgT[FIN] = 1.0
        xgTc = np.ascontiguousarray(
            xgT.reshape(FIN + 1, NEC, 128).transpose(0, 1, 2).reshape(FIN + 1, EP))
        dstcol = np.ascontiguousarray(colp.astype(np.float32).reshape(-1, 128).T)
        eaT = np.ascontiguousarray(
            eap.reshape(NEC, 128, FE + 1).transpose(2, 0, 1).reshape(FE + 1, EP))

        im = dict(shared)
        im.update(xTg=xTg, Gp=Gp, GTp=GTp, degones=degones,
                  xgT=xgTc, dstcol=dstcol, eaT=eaT)
        in_maps.append(im)

    return in_maps, EPC, NEC


def _build(nc, tc, EPC, NEC):
    """Emit one core's program (identical across cores; data differs)."""
    NZC = NPAD // 512
    NGG = GRID // 512
    NGROUP = NEC // GCHUNKS

    def dram_in(name, shape, dt):
        return nc.dram_tensor(name, list(shape), dt, kind="ExternalInput")

    xTg_d = dram_in("xTg", (FIN + 1, GRID), F16)
    xgT_d = dram_in("xgT", (FIN + 1, NEC * 128), F16)
    W0c_d = dram_in("W0c", (FIN + 1, D), F16)
    Wm1h_hi_d = dram_in("Wm1h_hi", (128, D), F16)
    Wm1h_lo_d = dram_in("Wm1h_lo", (128, D), F16)
    Wm1ec_d = dram_in("Wm1ec", (FE + 1, D), F16)
    Wm2_hi_d = dram_in("Wm2_hi", (128, D), F16)
    Wm2_lo_d = dram_in("Wm2_lo", (128, D), F16)
    Wr_hi_d = dram_in("Wr_hi", (128, D), F16)
    Wr_lo_d = dram_in("Wr_lo", (128, D), F16)
    bmbr_d = dram_in("bmbr", (2, D), F16)
    Wih_d = dram_in("Wih", (128, 4, 1024), F16)
    Whh_d = dram_in("Whh", (128, 2, 1024), F16)
    blr_d = dram_in("blr", (1, 1024), F16)
    W1p_d = dram_in("W1p", (128, 4, 2, 128), F16)
    b1c_d = dram_in("b1c", (128, 2), F32)
    W2s_d = dram_in("W2s", (128, 2), F16)
    b2t_d = dram_in("b2t", (1, 1), F16)
    ones1_d = dram_in("ones1", (1, 64), F16)
    iota_d = dram_in("iota_row", (128, 128), F16)
    ident_d = dram_in("ident", (128, 128), F16)
    Gp_d = dram_in("Gp", (128, NCH * GPC), F16)
    GTp_d = dram_in("GTp", (64, GRID), F16)
    degones_d = dram_in("degones", (2, GRID), F16)
    eaT_d = dram_in("eaT", (FE + 1, NEC * 128), F16)
    dstcol_d = dram_in("dstcol", (128, NEC), F32)

    y_d = nc.dram_tensor("y", [64, 1], F32, kind="ExternalOutput")

    def sb(name, shape, dt):
        return nc.alloc_sbuf_tensor(name, list(shape), dt).ap()

    s_w0 = sb("s_w0", (FIN + 1, D), F16)
    s_wm1hi = sb("s_wm1hi", (128, D), F16)
    s_wm1lo = sb("s_wm1lo", (128, D), F16)
    s_wm1ec = sb("s_wm1ec", (FE + 1, D), F16)
    s_wm2hi = sb("s_wm2hi", (128, D), F16)
    s_wm2lo = sb("s_wm2lo", (128, D), F16)
    s_wrhi = sb("s_wrhi", (128, D), F16)
    s_wrlo = sb("s_wrlo", (128, D), F16)
    s_bmbr = sb("s_bmbr", (2, D), F16)
    s_wih = sb("s_wih", (128, 4, 1024), F16)
    s_whh = sb("s_whh", (128, 2, 1024), F16)
    s_blr = sb("s_blr", (1, 1024), F16)
    s_w1 = sb("s_w1", (128, 4, 2, 128), F16)
    s_b1 = sb("s_b1", (128, 2), F32)
    s_w2 = sb("s_w2", (128, 2), F16)
    s_b2 = sb("s_b2", (1, 1), F16)
    s_ones1 = sb("s_ones1", (1, 64), F16)
    s_iota = sb("s_iota", (128, 128), F16)
    s_ident = sb("s_ident", (128, 128), F16)
    s_G = sb("s_G", (128, NCH, GPC), F16)
    s_GT = sb("s_GT", (64, GRID), F16)
    s_dego = sb("s_dego", (2, GRID), F16)
    s_dstcol = sb("s_dstcol", (128, NEC), F32)
    s_h0g_hi = sb("s_h0g_hi", (128, GRID), F16)
    s_h0g_lo = sb("s_h0g_lo", (128, GRID), F16)
    s_out = sb("s_out", (128, NCH, D + 1), F16)
    s_e = sb("s_e", (128, NCH), F32)
    s_wt = sb("s_wt", (128, NCH), F32)
    s_hT = [sb(f"s_hT{i}", (128, 64), F16) for i in range(2)]
    s_rT = [sb(f"s_rT{i}", (128, 64), F16) for i in range(2)]
    s_cu = sb("s_cu", (64, D), F32)
    s_hh16 = sb("s_hh16", (64, D), F16)
    s_y1 = [sb(f"s_y1_{i}", (128, 64), F16) for i in range(2)]
    s_yo = sb("s_yo", (64, 1), F32)

    dma = nc.sync.dma_start
    V, A, T, GP = nc.vector, nc.scalar, nc.tensor, nc.gpsimd

    for s, d in [(s_w0, W0c_d), (s_wm1hi, Wm1h_hi_d), (s_wm1lo, Wm1h_lo_d),
                 (s_wm1ec, Wm1ec_d), (s_wm2hi, Wm2_hi_d), (s_wm2lo, Wm2_lo_d),
                 (s_wrhi, Wr_hi_d), (s_wrlo, Wr_lo_d), (s_bmbr, bmbr_d),
                 (s_wih, Wih_d), (s_whh, Whh_d), (s_blr, blr_d),
                 (s_w1, W1p_d), (s_b1, b1c_d), (s_w2, W2s_d), (s_b2, b2t_d),
                 (s_ones1, ones1_d), (s_iota, iota_d), (s_ident, ident_d),
                 (s_GT, GTp_d), (s_dego, degones_d),
                 (s_dstcol, dstcol_d)]:
        dma(s[:], d[:])
    dma(s_G[:], Gp_d[:].rearrange("p (c g) -> p c g", g=GPC))

    V.memset(s_out[:, :, D:D + 1], 1.0)
    for t_ in (*s_hT, *s_rT):
        V.memset(t_[:], 0.0)
    V.memset(s_cu[:], 0.0)

    # ============ P1: grid h0T (resident, feeds the root update) ============
    with tc.tile_pool(name="p1ps", bufs=2, space="PSUM") as pp, \
         tc.tile_pool(name="p1sb", bufs=3) as ps:
        for cg in range(NGG):
            sl = slice(cg * 512, (cg + 1) * 512)
            xin = ps.tile([FIN + 1, 512], F16, tag="xin")
            dma(xin[:], xTg_d[:, sl])
            ph = pp.tile([128, 512], F32, tag="h0hi")
            pl = pp.tile([128, 512], F32, tag="h0lo")
            T.matmul(ph[:], s_w0[:, 0:128], xin[:])
            T.matmul(pl[:], s_w0[:, 128:256], xin[:])
            A.activation(s_h0g_hi[:, sl], ph[:], AF.Relu)
            V.tensor_relu(s_h0g_lo[:, sl], pl[:])

    # ============ P2: edge pipeline + segment sum + root update =============
    with tc.tile_pool(name="p2zg", bufs=3) as pzg, \
         tc.tile_pool(name="p2ea", bufs=3) as pea, \
         tc.tile_pool(name="p2sb", bufs=5) as ps2, \
         tc.tile_pool(name="p2ags", bufs=2) as pag, \
         tc.tile_pool(name="p2eaw", bufs=2, space="PSUM") as peaw, \
         tc.tile_pool(name="p2tp", bufs=3, space="PSUM") as ptp, \
         tc.tile_pool(name="p2agg", bufs=1, space="PSUM") as pagg, \
         tc.tile_pool(name="p2out", bufs=1, space="PSUM") as pout:
        for g in range(NGROUP):
            xgt = pzg.tile([FIN + 1, GCHUNKS * 128], F16, tag="xgt")
            dma(xgt[:], xgT_d[:, g * GCHUNKS * 128:(g + 1) * GCHUNKS * 128])
            eat = pea.tile([FE + 1, GCHUNKS * 128], F16, tag="eat")
            dma(eat[:], eaT_d[:, g * GCHUNKS * 128:(g + 1) * GCHUNKS * 128])
            for j2 in range(GCHUNKS // 2):
                psl = slice(j2 * 256, (j2 + 1) * 256)
                tp = ptp.tile([128, 2, 256], F32, tag="tp")
                T.matmul(tp[:, 0, :], s_w0[:, 0:128], xgt[:, psl])
                T.matmul(tp[:, 1, :], s_w0[:, 128:256], xgt[:, psl])
                t16 = ps2.tile([128, 2, 256], F16, tag="t16")
                A.activation(t16[:, 0, :], tp[:, 0, :], AF.Relu)
                V.tensor_relu(t16[:, 1, :], tp[:, 1, :])
                for h in range(2):
                    j = j2 * 2 + h
                    i = g * GCHUNKS + j
                    c, jj = i // EPC, i % EPC
                    esl = slice(j * 128, (j + 1) * 128)
                    hsl = slice(h * 128, (h + 1) * 128)
                    S = ps2.tile([128, 128], F16, tag="S")
                    V.tensor_scalar(S[:], s_iota[:], s_dstcol[:, i:i + 1], None,
                                    op0=ALU.is_equal)
                    pe_ = peaw.tile([128, D], F32, tag="eaw")
                    T.matmul(pe_[:], eat[:, esl], s_wm1ec[:],
                             start=True, stop=False)
                    T.matmul(pe_[:], t16[:, 0, hsl], s_wm1hi[:],
                             start=False, stop=False)
                    T.matmul(pe_[:], t16[:, 1, hsl], s_wm1lo[:],
                             start=False, stop=True)
                    m1 = ps2.tile([128, D], F16, tag="m1")
                    A.activation(m1[:], pe_[:], AF.Relu)
                    if jj == 0:
                        agh = pagg.tile([128, 128], F32, tag="agh")
                        agl = pagg.tile([128, 128], F32, tag="agl")
                    T.matmul(agh[:], m1[:, 0:128], S[:],
                             start=(jj == 0), stop=(jj == EPC - 1))
                    T.matmul(agl[:], m1[:, 128:256], S[:],
                             start=(jj == 0), stop=(jj == EPC - 1))
                    if jj == EPC - 1:
                        ags = pag.tile([128, 2, 128], F16, tag="ags")
                        V.tensor_copy(ags[:, 0, :], agh[:])
                        A.activation(ags[:, 1, :], agl[:], AF.Copy)
                        po = pout.tile([128, D], F32, tag="po")
                        csl = slice(c * 128, (c + 1) * 128)
                        T.matmul(po[:], ags[:, 0, :], s_wm2hi[:], start=True, stop=False)
                        T.matmul(po[:], ags[:, 1, :], s_wm2lo[:], start=False, stop=False)
                        T.matmul(po[:], s_h0g_hi[:, csl], s_wrhi[:], start=False, stop=False)
                        T.matmul(po[:], s_h0g_lo[:, csl], s_wrlo[:], start=False, stop=False)
                        T.matmul(po[:], s_dego[:, csl], s_bmbr[:], start=False, stop=True)
                        A.activation(s_out[:, c, 0:D], po[:], AF.Relu)

    # ============ P3: Set2Set (3 steps) + readout ===========================
    with tc.tile_pool(name="p3ps", bufs=2, space="PSUM") as pp3, \
         tc.tile_pool(name="p3p1", bufs=1, space="PSUM") as pq3, \
         tc.tile_pool(name="p3g", bufs=1, space="PSUM") as pg3, \
         tc.tile_pool(name="p3sb", bufs=2) as ps3:
        for step in range(S2S_STEPS := 3):
            g0p = pg3.tile([64, 512], F32, tag="g0")
            g1p = pg3.tile([64, 512], F32, tag="g1")
            for half, gp in ((0, g0p), (1, g1p)):
                nsl = slice(half * 512, (half + 1) * 512)
                T.matmul(gp[:], s_ones1[:], s_blr[:, nsl], start=True, stop=False)
                for kk in range(4):
                    lhs = (s_hT + s_rT)[kk]
                    T.matmul(gp[:], lhs[:], s_wih[:, kk, nsl],
                             start=False, stop=False)
                for kk in range(2):
                    T.matmul(gp[:], s_hT[kk][:], s_whh[:, kk, nsl],
                             start=False, stop=(kk == 1))
            ti = ps3.tile([64, D], F32, tag="ti")
            tf = ps3.tile([64, D], F32, tag="tf")
            tg = ps3.tile([64, D], F32, tag="tg")
            to = ps3.tile([64, D], F32, tag="to")
            A.activation(ti[:], g0p[:, 0:256], AF.Tanh, scale=0.5)
            A.activation(tf[:], g0p[:, 256:512], AF.Tanh, scale=0.5)
            A.activation(tg[:], g1p[:, 0:256], AF.Tanh)
            A.activation(to[:], g1p[:, 256:512], AF.Tanh, scale=0.5)
            a2 = ps3.tile([64, D], F32, tag="a2")
            bv = ps3.tile([64, D], F32, tag="bv")
            V.scalar_tensor_tensor(a2[:], tf[:], 1.0, s_cu[:], ALU.add, ALU.mult)
            V.scalar_tensor_tensor(bv[:], ti[:], 1.0, tg[:], ALU.add, ALU.mult)
            V.scalar_tensor_tensor(s_cu[:], a2[:], 0.5, bv[:], ALU.mult, ALU.add)
            th = ps3.tile([64, D], F32, tag="th")
            A.activation(th[:], s_cu[:], AF.Tanh, scale=0.5)
            hh = ps3.tile([64, D], F32, tag="hh")
            V.scalar_tensor_tensor(hh[:], to[:], 1.0, th[:], ALU.add, ALU.mult)
            A.activation(s_hh16[:], hh[:], AF.Copy)
            for mth in range(2):
                ptr = pq3.tile([128, 64], F16, tag="ptr")
                T.transpose(ptr[:], s_hh16[:, mth * 128:(mth + 1) * 128],
                            s_ident[0:64, 0:64])
                V.tensor_copy(s_hT[mth][:], ptr[:])
            # attention: e, w=exp(e), r = (sum w*out)/(sum w)
            for c in range(NCH):
                csl = slice(c * 128, (c + 1) * 128)
                hb = pp3.tile([128, D], F32, tag="hb")
                T.matmul(hb[:], s_GT[:, csl], s_hh16[:])
                scr = ps3.tile([128, D], F32, tag="scr")
                V.scalar_tensor_tensor(scr[:], s_out[:, c, 0:D], 0.5, hb[:],
                                       ALU.mult, ALU.mult,
                                       accum_out=s_e[:, c:c + 1])
            A.activation(s_wt[:], s_e[:], AF.Exp)
            rw = pg3.tile([64, 257], F32, tag="rw")
            for c in range(NCH):
                gw = ps3.tile([128, 64], F16, tag="gw")
                V.tensor_scalar(gw[:], s_G[:, c, :], s_wt[:, c:c + 1], None,
                                op0=ALU.mult)
                T.matmul(rw[:], gw[:], s_out[:, c, :],
                         start=(c == 0), stop=(c == NCH - 1))
            rr = ps3.tile([64, 1], F32, tag="rr")
            V.reciprocal(rr[:], rw[:, 256:257])
            rf = ps3.tile([64, D], F16, tag="rf")
            V.tensor_scalar(rf[:], rw[:, 0:256], rr[:], None, op0=ALU.mult)
            for mth in range(2):
                ptr = pq3.tile([128, 64], F16, tag="ptr")
                T.transpose(ptr[:], rf[:, mth * 128:(mth + 1) * 128],
                            s_ident[0:64, 0:64])
                V.tensor_copy(s_rT[mth][:], ptr[:])
        # readout
        for mth in range(2):
            yp = pq3.tile([128, 64], F32, tag="yp")
            for kk in range(4):
                T.matmul(yp[:], s_w1[:, kk, mth, :], (s_hT + s_rT)[kk][:],
                         start=(kk == 0), stop=(kk == 3))
            A.activation(s_y1[mth][:], yp[:], AF.Relu, bias=s_b1[:, mth:mth + 1])
        ypo = pq3.tile([64, 1], F32, tag="ypo")
        T.matmul(ypo[:], s_y1[0][:], s_w2[:, 0:1], start=True, stop=False)
        T.matmul(ypo[:], s_y1[1][:], s_w2[:, 1:2], start=False, stop=False)
        T.matmul(ypo[:], s_ones1[:], s_b2[:], start=False, stop=True)
        V.tensor_copy(s_yo[:], ypo[:])
        dma(y_d[:], s_yo[:])


_CACHE = {}


def _get_compiled(EPC, NEC):
    key = (EPC, NEC)
    if key not in _CACHE:
        nc = bacc.Bacc("TRN2", target_bir_lowering=False, debug=False,
                       num_devices=N_CORES)
        with tile.TileContext(nc) as tc:
            _build(nc, tc, EPC, NEC)
        nc.compile()
        _CACHE[key] = nc
    return _CACHE[key]


def kernel(**inputs) -> np.ndarray:
    in_maps, EPC, NEC = _host_prep(inputs)
    nc = _get_compiled(EPC, NEC)
    res = run_bass_kernel_spmd(nc, in_maps, list(range(N_CORES)))
    y = np.concatenate([res.results[k]["y"].reshape(-1) for k in range(N_CORES)])
    return y.astype(np.float32)



# revision 24
# speedup vs baseline: 103.5769x; 103.5769x over previous
"""Trainium2 Bass kernel for nn_DMPNN_Change_678604832935 (8-core SPMD DMPNN+Set2Set).

Sharding: each core owns 64 consecutive graphs (batch is sorted) plus all edges
whose dst node falls in those graphs — so segment_sum is core-local and no
collectives are needed.  Nodes are packed DENSELY into 128-slot windows
(~30 windows/core instead of one window per graph), and windows are sorted by
edge count so the per-window chunk counts Kc align across cores (the program
is SPMD-shared; Kc is compile-time).  Per-edge h0 is recomputed from
host-pre-gathered x[src] (cheaper than on-chip gather); since segment_sum is
linear and sits between the Wm2 matmul and the root update, m@Wm2 is folded to
the node side (16x fewer FLOPs), with deg(n)*bm2 as a rank-1 correction.
Set2Set attention/segment ops are matmuls against one-hot slot<->graph maps,
so they are agnostic to graphs spanning windows.  Softmax uses unnormalized
exp (|e|<~8, validated); sigmoid is synthesized from tanh so the whole kernel
uses one ACT table set.
"""

import os
import sys

for _p in ("/opt/trn_rl_repo", "/root/.axon_site/_ro/trn_rl_repo"):
    if os.path.isdir(_p) and _p not in sys.path:
        sys.path.append(_p)

import numpy as np

import concourse.bass as bass
import concourse.bacc as bacc
import concourse.mybir as mybir
import concourse.tile as tile
from concourse.bass_utils import run_bass_kernel_spmd

F16 = mybir.dt.float16
F32 = mybir.dt.float32
AF = mybir.ActivationFunctionType
ALU = mybir.AluOpType

N_NODES = 30000
FIN = 25
FE = 14
D = 256
N_GRAPHS = 512
N_CORES = 8
GPC = N_GRAPHS // N_CORES      # graphs per core
S2S_STEPS = 3
GCHUNKS = 16                   # edge chunks per input-stream DMA


def _f16(a):
    return np.ascontiguousarray(np.asarray(a, np.float32).astype(np.float16))


def _host_prep(inp):
    """Pure index/layout/dtype work: build per-core input maps."""
    x = np.asarray(inp["x"], np.float32)
    ea = np.asarray(inp["edge_attr"], np.float32)
    ei = np.asarray(inp["edge_index"])
    batch = np.asarray(inp["batch"]).astype(np.int64)
    src_all = np.asarray(ei[0], np.int64)
    dst_all = np.asarray(ei[1], np.int64)

    counts = np.bincount(batch, minlength=N_GRAPHS)
    starts = np.zeros(N_GRAPHS + 1, np.int64)
    np.cumsum(counts, out=starts[1:])
    core_ns = starts[np.arange(N_CORES) * GPC]
    core_ne = starts[(np.arange(N_CORES) + 1) * GPC]

    NCH = int(np.ceil((core_ne - core_ns).max() / 128.0))
    GRID = NCH * 128
    GRIDP = ((GRID + 511) // 512) * 512

    # Per-core window sort (by edge count, desc) so chunk counts align
    # across cores; Kc[c] is then the cross-core max for rank-c windows.
    ranks, ew_sorted = [], np.zeros((N_CORES, NCH), np.int64)
    for k in range(N_CORES):
        ns, ne = int(core_ns[k]), int(core_ne[k])
        m = (dst_all >= ns) & (dst_all < ne)
        ew = np.bincount((dst_all[m] - ns) // 128, minlength=NCH)
        order = np.argsort(-ew, kind="stable")
        rank = np.empty(NCH, np.int64)
        rank[order] = np.arange(NCH)
        ranks.append(rank)
        ew_sorted[k] = ew[order]
    Kc = np.maximum(1, np.ceil(ew_sorted.max(axis=0) / 128.0).astype(np.int64))
    NEC = int(Kc.sum())
    Kc[-1] += (-NEC) % GCHUNKS
    NEC = int(Kc.sum())
    Coff = np.zeros(NCH + 1, np.int64)
    np.cumsum(Kc, out=Coff[1:])
    EP = NEC * 128

    W0 = np.asarray(inp["W0"], np.float32); b0 = np.asarray(inp["b0"], np.float32)
    Wm1 = np.asarray(inp["Wm1"], np.float32); bm1 = np.asarray(inp["bm1"], np.float32)
    Wm2 = np.asarray(inp["Wm2"], np.float32); bm2 = np.asarray(inp["bm2"], np.float32)
    Wr = np.asarray(inp["Wr"], np.float32); br = np.asarray(inp["br"], np.float32)
    Wih = np.asarray(inp["Wih"], np.float32); Whh = np.asarray(inp["Whh"], np.float32)
    bl = np.asarray(inp["bl"], np.float32)
    W1 = np.asarray(inp["W1"], np.float32); b1 = np.asarray(inp["b1"], np.float32)
    W2 = np.asarray(inp["W2"], np.float32); b2 = np.asarray(inp["b2"], np.float32)

    W0c = _f16(np.concatenate([W0, b0[None, :]], 0))            # [26, 256]
    Wm1h = _f16(Wm1[:D])
    Wm1ec = _f16(np.concatenate([Wm1[D:], bm1[None, :]], 0))    # [15, 256]
    Wih_s = Wih.copy(); Wih_s[:D] *= 0.5                        # h state kept as 2h
    W1_s = W1.copy(); W1_s[:D] *= 0.5
    W1p = np.zeros((128, 4, 2, 128), np.float16)
    for kk in range(4):
        for m in range(2):
            W1p[:, kk, m, :] = _f16(W1_s[kk * 128:(kk + 1) * 128,
                                         m * 128:(m + 1) * 128])
    b1c = np.zeros((128, 2), np.float32)
    b1c[:, 0] = b1[:128]; b1c[:, 1] = b1[128:]
    W2s = np.zeros((128, 2), np.float16)
    W2s[:, 0] = _f16(W2[:128, 0]); W2s[:, 1] = _f16(W2[128:, 0])

    shared = dict(
        W0c=W0c,
        Wm1h_hi=_f16(Wm1h[:128]), Wm1h_lo=_f16(Wm1h[128:]),
        Wm1ec=Wm1ec,
        Wm2_hi=_f16(Wm2[:128]), Wm2_lo=_f16(Wm2[128:]),
        Wr_hi=_f16(Wr[:128]), Wr_lo=_f16(Wr[128:]),
        bmbr=_f16(np.stack([bm2, br], 0)),
        Wih=np.ascontiguousarray(_f16(Wih_s).reshape(4, 128, 1024).transpose(1, 0, 2)),
        Whh=np.ascontiguousarray(_f16(Whh * 0.5).reshape(2, 128, 1024).transpose(1, 0, 2)),
        blr=_f16(bl[None, :]),
        W1p=W1p, b1c=b1c, W2s=W2s, b2t=_f16(b2.reshape(1, 1)),
        ones1=np.ones((1, 64), np.float16),
        ident=np.eye(128, dtype=np.float16),
    )

    in_maps = []
    for k in range(N_CORES):
        ns, ne = int(core_ns[k]), int(core_ne[k])
        n_core = ne - ns
        rank = ranks[k]
        local = np.arange(n_core)
        slot = rank[local // 128] * 128 + (local % 128)   # node -> grid slot
        gr = batch[ns:ne] - k * GPC                        # node -> graph row

        xTg = np.zeros((FIN + 1, GRIDP), np.float16)
        xTg[:FIN, slot] = _f16(x[ns:ne].T)
        xTg[FIN, :] = 1.0

        Gp = np.zeros((128, NCH * GPC), np.float16)
        Gp[slot % 128, (slot // 128) * GPC + gr] = 1.0
        GTp = np.zeros((64, GRID), np.float16)
        GTp[gr, slot] = 1.0

        m = (dst_all >= ns) & (dst_all < ne)
        e_src = src_all[m]; e_ea = ea[m]
        e_slot = rank[(dst_all[m] - ns) // 128] * 128 + (dst_all[m] - ns) % 128
        e_win = e_slot // 128
        order = np.argsort(e_win, kind="stable")
        e_src, e_slot, e_ea, e_win = (e_src[order], e_slot[order],
                                      e_ea[order], e_win[order])

        deg = np.zeros(GRID, np.float32)
        np.add.at(deg, e_slot, 1.0)
        degones = np.zeros((2, GRID), np.float16)
        degones[0] = deg.astype(np.float16); degones[1] = 1.0

        srcp = np.zeros(EP, np.int64)
        colp = np.full(EP, 255.0, np.float32)
        eap = np.zeros((EP, FE + 1), np.float16)
        cstart = np.searchsorted(e_win, np.arange(NCH + 1))
        for c in range(NCH):
            a, b = int(cstart[c]), int(cstart[c + 1])
            n_e = b - a
            assert n_e <= int(Kc[c]) * 128
            o = int(Coff[c]) * 128
            srcp[o:o + n_e] = e_src[a:b]
            colp[o:o + n_e] = (e_slot[a:b] % 128).astype(np.float32)
            eap[o:o + n_e, :FE] = _f16(e_ea[a:b])
            eap[o:o + n_e, FE] = 1.0

        xgT = np.empty((FIN + 1, EP), np.float16)
        xgT[:FIN] = _f16(x[srcp].T)
        xgT[FIN] = 1.0
        eaT = np.ascontiguousarray(
            eap.reshape(NEC, 128, FE + 1).transpose(2, 0, 1).reshape(FE + 1, EP))
        # scatter selection matrices, host-built (frees DVE + a dependency
        # rung per chunk): S[p, i*128+c] = 1 iff edge p of chunk i hits col c
        er = np.arange(EP)[colp < 128]
        Shost = np.zeros((128, EP), np.float16)
        Shost[er % 128, (er // 128) * 128 + colp[er].astype(np.int64)] = 1.0

        im = dict(shared)
        im.update(xTg=xTg, Gp=Gp, GTp=GTp, degones=degones,
                  xgT=np.ascontiguousarray(xgT), S=Shost, eaT=eaT)
        in_maps.append(im)

    return in_maps, NCH, tuple(int(v) for v in Kc)


def _build(nc, tc, NCH, Kc, rep=1):
    """Emit one core's program (identical across cores; data differs).

    rep>1 wraps the whole body in a hardware For_i loop (used only by the
    timing harness to amortize the ~100ms axon dispatch overhead)."""
    NEC = int(sum(Kc))
    GRID = NCH * 128
    GRIDP = ((GRID + 511) // 512) * 512
    NGG = GRIDP // 512
    NGROUP = NEC // GCHUNKS
    chunk_win = [c for c in range(NCH) for _ in range(Kc[c])]
    chunk_jj = [j for c in range(NCH) for j in range(Kc[c])]

    def dram_in(name, shape, dt):
        return nc.dram_tensor(name, list(shape), dt, kind="ExternalInput")

    xTg_d = dram_in("xTg", (FIN + 1, GRIDP), F16)
    xgT_d = dram_in("xgT", (FIN + 1, NEC * 128), F16)
    W0c_d = dram_in("W0c", (FIN + 1, D), F16)
    Wm1h_hi_d = dram_in("Wm1h_hi", (128, D), F16)
    Wm1h_lo_d = dram_in("Wm1h_lo", (128, D), F16)
    Wm1ec_d = dram_in("Wm1ec", (FE + 1, D), F16)
    Wm2_hi_d = dram_in("Wm2_hi", (128, D), F16)
    Wm2_lo_d = dram_in("Wm2_lo", (128, D), F16)
    Wr_hi_d = dram_in("Wr_hi", (128, D), F16)
    Wr_lo_d = dram_in("Wr_lo", (128, D), F16)
    bmbr_d = dram_in("bmbr", (2, D), F16)
    Wih_d = dram_in("Wih", (128, 4, 1024), F16)
    Whh_d = dram_in("Whh", (128, 2, 1024), F16)
    blr_d = dram_in("blr", (1, 1024), F16)
    W1p_d = dram_in("W1p", (128, 4, 2, 128), F16)
    b1c_d = dram_in("b1c", (128, 2), F32)
    W2s_d = dram_in("W2s", (128, 2), F16)
    b2t_d = dram_in("b2t", (1, 1), F16)
    ones1_d = dram_in("ones1", (1, 64), F16)
    ident_d = dram_in("ident", (128, 128), F16)
    Gp_d = dram_in("Gp", (128, NCH * GPC), F16)
    GTp_d = dram_in("GTp", (64, GRID), F16)
    degones_d = dram_in("degones", (2, GRID), F16)
    eaT_d = dram_in("eaT", (FE + 1, NEC * 128), F16)
    S_d = dram_in("S", (128, NEC * 128), F16)

    y_d = nc.dram_tensor("y", [64, 1], F32, kind="ExternalOutput")

    def sb(name, shape, dt):
        return nc.alloc_sbuf_tensor(name, list(shape), dt).ap()

    s_w0 = sb("s_w0", (FIN + 1, D), F16)
    s_wm1hi = sb("s_wm1hi", (128, D), F16)
    s_wm1lo = sb("s_wm1lo", (128, D), F16)
    s_wm1ec = sb("s_wm1ec", (FE + 1, D), F16)
    s_wm2hi = sb("s_wm2hi", (128, D), F16)
    s_wm2lo = sb("s_wm2lo", (128, D), F16)
    s_wrhi = sb("s_wrhi", (128, D), F16)
    s_wrlo = sb("s_wrlo", (128, D), F16)
    s_bmbr = sb("s_bmbr", (2, D), F16)
    s_wih = sb("s_wih", (128, 4, 1024), F16)
    s_whh = sb("s_whh", (128, 2, 1024), F16)
    s_blr = sb("s_blr", (1, 1024), F16)
    s_w1 = sb("s_w1", (128, 4, 2, 128), F16)
    s_b1 = sb("s_b1", (128, 2), F32)
    s_w2 = sb("s_w2", (128, 2), F16)
    s_b2 = sb("s_b2", (1, 1), F16)
    s_ones1 = sb("s_ones1", (1, 64), F16)
    s_ident = sb("s_ident", (128, 128), F16)
    s_G = sb("s_G", (128, NCH, GPC), F16)
    s_GT = sb("s_GT", (64, GRID), F16)
    s_dego = sb("s_dego", (2, GRID), F16)
    s_h0g_hi = sb("s_h0g_hi", (128, GRIDP), F16)
    s_h0g_lo = sb("s_h0g_lo", (128, GRIDP), F16)
    s_out = sb("s_out", (128, NCH, D + 1), F16)
    s_e = sb("s_e", (128, NCH), F32)
    s_wt = sb("s_wt", (128, NCH), F32)
    s_hT = [sb(f"s_hT{i}", (128, 64), F16) for i in range(2)]
    s_rT = [sb(f"s_rT{i}", (128, 64), F16) for i in range(2)]
    s_cu = sb("s_cu", (64, D), F32)
    s_hh16 = sb("s_hh16", (64, D), F16)
    s_y1 = [sb(f"s_y1_{i}", (128, 64), F16) for i in range(2)]
    s_yo = sb("s_yo", (64, 1), F32)

    dma = nc.sync.dma_start
    V, A, T, GP = nc.vector, nc.scalar, nc.tensor, nc.gpsimd

    def _emit():
        for s, d in [(s_w0, W0c_d), (s_wm1hi, Wm1h_hi_d), (s_wm1lo, Wm1h_lo_d),
                     (s_wm1ec, Wm1ec_d), (s_wm2hi, Wm2_hi_d), (s_wm2lo, Wm2_lo_d),
                     (s_wrhi, Wr_hi_d), (s_wrlo, Wr_lo_d), (s_bmbr, bmbr_d),
                     (s_wih, Wih_d), (s_whh, Whh_d), (s_blr, blr_d),
                     (s_w1, W1p_d), (s_b1, b1c_d), (s_w2, W2s_d), (s_b2, b2t_d),
                     (s_ones1, ones1_d), (s_ident, ident_d),
                     (s_GT, GTp_d), (s_dego, degones_d)]:
            dma(s[:], d[:])
        dma(s_G[:], Gp_d[:].rearrange("p (c g) -> p c g", g=GPC))

        V.memset(s_out[:, :, D:D + 1], 1.0)
        for t_ in (*s_hT, *s_rT):
            V.memset(t_[:], 0.0)
        V.memset(s_cu[:], 0.0)

        # ======== P1: grid h0T (resident, feeds the root update) ========
        with tc.tile_pool(name="p1ps", bufs=2, space="PSUM") as pp, \
             tc.tile_pool(name="p1sb", bufs=3) as ps:
            for cg in range(NGG):
                sl = slice(cg * 512, (cg + 1) * 512)
                xin = ps.tile([FIN + 1, 512], F16, tag="xin")
                dma(xin[:], xTg_d[:, sl])
                ph = pp.tile([128, 512], F32, tag="h0hi")
                pl = pp.tile([128, 512], F32, tag="h0lo")
                T.matmul(ph[:], s_w0[:, 0:128], xin[:])
                T.matmul(pl[:], s_w0[:, 128:256], xin[:])
                A.activation(s_h0g_hi[:, sl], ph[:], AF.Relu)
                V.tensor_relu(s_h0g_lo[:, sl], pl[:])

        # ======== P2: edge pipeline + segment sum + root update =========
        with tc.tile_pool(name="p2zg", bufs=3) as pzg, \
             tc.tile_pool(name="p2ea", bufs=3) as pea, \
             tc.tile_pool(name="p2S", bufs=3) as psS, \
             tc.tile_pool(name="p2sb", bufs=5) as ps2, \
             tc.tile_pool(name="p2ags", bufs=2) as pag, \
             tc.tile_pool(name="p2eaw", bufs=2, space="PSUM") as peaw, \
             tc.tile_pool(name="p2tp", bufs=3, space="PSUM") as ptp, \
             tc.tile_pool(name="p2agg", bufs=2, space="PSUM") as pagg, \
             tc.tile_pool(name="p2out", bufs=1, space="PSUM") as pout:
            ag = None
            for g in range(NGROUP):
                xgt = pzg.tile([FIN + 1, GCHUNKS * 128], F16, tag="xgt")
                dma(xgt[:], xgT_d[:, g * GCHUNKS * 128:(g + 1) * GCHUNKS * 128])
                eat = pea.tile([FE + 1, GCHUNKS * 128], F16, tag="eat")
                dma(eat[:], eaT_d[:, g * GCHUNKS * 128:(g + 1) * GCHUNKS * 128])
                Sg = psS.tile([128, GCHUNKS * 128], F16, tag="Sg")
                dma(Sg[:], S_d[:, g * GCHUNKS * 128:(g + 1) * GCHUNKS * 128])
                for j2 in range(GCHUNKS // 2):
                    psl = slice(j2 * 256, (j2 + 1) * 256)
                    tp = ptp.tile([128, 2, 256], F32, tag="tp")
                    T.matmul(tp[:, 0, :], s_w0[:, 0:128], xgt[:, psl])
                    T.matmul(tp[:, 1, :], s_w0[:, 128:256], xgt[:, psl])
                    t16 = ps2.tile([128, 2, 256], F16, tag="t16")
                    A.activation(t16[:, 0, :], tp[:, 0, :], AF.Relu)
                    V.tensor_relu(t16[:, 1, :], tp[:, 1, :])
                    for h in range(2):
                        i = g * GCHUNKS + j2 * 2 + h
                        c, jj = chunk_win[i], chunk_jj[i]
                        last = jj == Kc[c] - 1
                        esl = slice((j2 * 2 + h) * 128, (j2 * 2 + h + 1) * 128)
                        hsl = slice(h * 128, (h + 1) * 128)
                        S = Sg[:, esl]
                        pe_ = peaw.tile([128, D], F32, tag="eaw")
                        T.matmul(pe_[:], eat[:, esl], s_wm1ec[:],
                                 start=True, stop=False)
                        T.matmul(pe_[:], t16[:, 0, hsl], s_wm1hi[:],
                                 start=False, stop=False)
                        T.matmul(pe_[:], t16[:, 1, hsl], s_wm1lo[:],
                                 start=False, stop=True)
                        m1 = ps2.tile([128, D], F16, tag="m1")
                        A.activation(m1[:], pe_[:], AF.Relu)
                        # hi/lo subtiles share one PSUM bank, so they form a
                        # single accumulation group: start only on the first
                        # matmul of the window, stop only on the very last.
                        if jj == 0:
                            ag = pagg.tile([128, 2, 128], F32, tag="ag")
                        T.matmul(ag[:, 0, :], m1[:, 0:128], S,
                                 start=(jj == 0), stop=False)
                        T.matmul(ag[:, 1, :], m1[:, 128:256], S,
                                 start=False, stop=last)
                        if last:
                            ags = pag.tile([128, 2, 128], F16, tag="ags")
                            V.tensor_copy(ags[:, 0, :], ag[:, 0, :])
                            A.activation(ags[:, 1, :], ag[:, 1, :], AF.Copy)
                            po = pout.tile([128, D], F32, tag="po")
                            csl = slice(c * 128, (c + 1) * 128)
                            T.matmul(po[:], ags[:, 0, :], s_wm2hi[:],
                                     start=True, stop=False)
                            T.matmul(po[:], ags[:, 1, :], s_wm2lo[:],
                                     start=False, stop=False)
                            T.matmul(po[:], s_h0g_hi[:, csl], s_wrhi[:],
                                     start=False, stop=False)
                            T.matmul(po[:], s_h0g_lo[:, csl], s_wrlo[:],
                                     start=False, stop=False)
                            T.matmul(po[:], s_dego[:, csl], s_bmbr[:],
                                     start=False, stop=True)
                            A.activation(s_out[:, c, 0:D], po[:], AF.Relu)

        # ======== P3: Set2Set (3 steps) + readout =======================
        with tc.tile_pool(name="p3ps", bufs=2, space="PSUM") as pp3, \
             tc.tile_pool(name="p3p1", bufs=1, space="PSUM") as pq3, \
             tc.tile_pool(name="p3g", bufs=1, space="PSUM") as pg3, \
             tc.tile_pool(name="p3sb", bufs=2) as ps3:
            for step in range(S2S_STEPS):
                g0p = pg3.tile([64, 512], F32, tag="g0")
                g1p = pg3.tile([64, 512], F32, tag="g1")
                for half, gp in ((0, g0p), (1, g1p)):
                    nsl = slice(half * 512, (half + 1) * 512)
                    T.matmul(gp[:], s_ones1[:], s_blr[:, nsl], start=True, stop=False)
                    for kk in range(4):
                        lhs = (s_hT + s_rT)[kk]
                        T.matmul(gp[:], lhs[:], s_wih[:, kk, nsl],
                                 start=False, stop=False)
                    for kk in range(2):
                        T.matmul(gp[:], s_hT[kk][:], s_whh[:, kk, nsl],
                                 start=False, stop=(kk == 1))
                ti = ps3.tile([64, D], F32, tag="ti")
                tf = ps3.tile([64, D], F32, tag="tf")
                tg = ps3.tile([64, D], F32, tag="tg")
                to = ps3.tile([64, D], F32, tag="to")
                A.activation(ti[:], g0p[:, 0:256], AF.Tanh, scale=0.5)
                A.activation(tf[:], g0p[:, 256:512], AF.Tanh, scale=0.5)
                A.activation(tg[:], g1p[:, 0:256], AF.Tanh)
                A.activation(to[:], g1p[:, 256:512], AF.Tanh, scale=0.5)
                a2 = ps3.tile([64, D], F32, tag="a2")
                bv = ps3.tile([64, D], F32, tag="bv")
                V.scalar_tensor_tensor(a2[:], tf[:], 1.0, s_cu[:], ALU.add, ALU.mult)
                V.scalar_tensor_tensor(bv[:], ti[:], 1.0, tg[:], ALU.add, ALU.mult)
                V.scalar_tensor_tensor(s_cu[:], a2[:], 0.5, bv[:], ALU.mult, ALU.add)
                th = ps3.tile([64, D], F32, tag="th")
                A.activation(th[:], s_cu[:], AF.Tanh, scale=0.5)
                hh = ps3.tile([64, D], F32, tag="hh")
                V.scalar_tensor_tensor(hh[:], to[:], 1.0, th[:], ALU.add, ALU.mult)
                A.activation(s_hh16[:], hh[:], AF.Copy)
                for mth in range(2):
                    ptr = pq3.tile([128, 64], F16, tag="ptr")
                    T.transpose(ptr[:], s_hh16[:, mth * 128:(mth + 1) * 128],
                                s_ident[0:64, 0:64])
                    V.tensor_copy(s_hT[mth][:], ptr[:])
                # attention: e, w=exp(e), r = (sum w*out)/(sum w)
                for c in range(NCH):
                    csl = slice(c * 128, (c + 1) * 128)
                    hb = pp3.tile([128, D], F32, tag="hb")
                    T.matmul(hb[:], s_GT[:, csl], s_hh16[:])
                    scr = ps3.tile([128, D], F32, tag="scr")
                    V.scalar_tensor_tensor(scr[:], s_out[:, c, 0:D], 0.5, hb[:],
                                           ALU.mult, ALU.mult,
                                           accum_out=s_e[:, c:c + 1])
                A.activation(s_wt[:], s_e[:], AF.Exp)
                rw = pg3.tile([64, 257], F32, tag="rw")
                for c in range(NCH):
                    gw = ps3.tile([128, 64], F16, tag="gw")
                    V.tensor_scalar(gw[:], s_G[:, c, :], s_wt[:, c:c + 1], None,
                                    op0=ALU.mult)
                    T.matmul(rw[:], gw[:], s_out[:, c, :],
                             start=(c == 0), stop=(c == NCH - 1))
                rr = ps3.tile([64, 1], F32, tag="rr")
                V.reciprocal(rr[:], rw[:, 256:257])
                rf = ps3.tile([64, D], F16, tag="rf")
                V.tensor_scalar(rf[:], rw[:, 0:256], rr[:], None, op0=ALU.mult)
                for mth in range(2):
                    ptr = pq3.tile([128, 64], F16, tag="ptr")
                    T.transpose(ptr[:], rf[:, mth * 128:(mth + 1) * 128],
                                s_ident[0:64, 0:64])
                    V.tensor_copy(s_rT[mth][:], ptr[:])
            # readout
            for mth in range(2):
                yp = pq3.tile([128, 64], F32, tag="yp")
                for kk in range(4):
                    T.matmul(yp[:], s_w1[:, kk, mth, :], (s_hT + s_rT)[kk][:],
                             start=(kk == 0), stop=(kk == 3))
                A.activation(s_y1[mth][:], yp[:], AF.Relu, bias=s_b1[:, mth:mth + 1])
            ypo = pq3.tile([64, 1], F32, tag="ypo")
            T.matmul(ypo[:], s_y1[0][:], s_w2[:, 0:1], start=True, stop=False)
            T.matmul(ypo[:], s_y1[1][:], s_w2[:, 1:2], start=False, stop=False)
            T.matmul(ypo[:], s_ones1[:], s_b2[:], start=False, stop=True)
            V.tensor_copy(s_yo[:], ypo[:])
            dma(y_d[:], s_yo[:])

    if rep > 1:
        with tc.For_i(0, rep, 1):
            _emit()
    else:
        _emit()


_CACHE = {}


def _get_compiled(NCH, Kc, rep=1):
    key = (NCH, tuple(Kc), rep)
    if key not in _CACHE:
        nc = bacc.Bacc("TRN2", target_bir_lowering=False, debug=False,
                       num_devices=N_CORES)
        with tile.TileContext(nc) as tc:
            _build(nc, tc, NCH, tuple(Kc), rep=rep)
        nc.compile()
        _CACHE[key] = nc
    return _CACHE[key]


def kernel(**inputs) -> np.ndarray:
    in_maps, NCH, Kc = _host_prep(inputs)
    nc = _get_compiled(NCH, Kc)
    res = run_bass_kernel_spmd(nc, in_maps, list(range(N_CORES)))
    y = np.concatenate([res.results[k]["y"].reshape(-1) for k in range(N_CORES)])
    return y.astype(np.float32)
